# revision 13
# baseline (speedup 1.0000x reference)
"""GatedAttention Trainium2 kernel, 8-way tensor-parallel over heads.

Reference computation (B=1, S=2048, D=2048, H=16 heads, Hd=128):
  q,k,v = x @ {q,k,v}_w.T  (per-head split)
  scores = (q @ k.T) / sqrt(Hd), causal mask, softmax
  av = attn @ v
  gate = sigmoid(q @ gate_w.T + gate_b)       (per-head)
  y = concat_heads(av * gate) @ o_w.T

Sharding: 2 heads per core (column-parallel QKV/gate). o_proj is computed
as a per-core PARTIAL product over the core's 256 local features into the
full [D, S] output; the host sums the 8 partials. No collectives at all —
each core's program is fully independent, so no core ever waits on
another.

All matmuls run on the PE in bf16 with fp32 PSUM accumulation. Softmax runs
without max-subtraction (scores are small by construction). Attention works
in the transposed [key, query] layout so no on-chip transposes are needed;
exps run 1024 columns at a time (two key chunks share one two-bank PSUM
tile) to halve the ACT instruction count, and exp row-sums accumulate on
the DVE so the PE only pays one ones-matmul per query block. o_proj chunks
are interleaved into the second head's attention stream, filling the PE
slack left by the ACT-paced softmax. A short burst of dummy matmuls at t=0
warms the PE clock gate (HAM) while the first input DMAs are in flight.
"""

import numpy as np
import ml_dtypes

import concourse.bass as bass
import concourse.mybir as mybir
import concourse.tile as tile
from concourse import bacc
from concourse.bass_utils import run_bass_kernel_spmd

BF16 = ml_dtypes.bfloat16
F32 = mybir.dt.float32
BF = mybir.dt.bfloat16
AF = mybir.ActivationFunctionType

N_CORES = 8
S = 2048          # sequence length
D = 2048          # model dim
H = 16            # total heads
HD = 128          # head dim
HPC = H // N_CORES                   # heads per core: 2
E = HPC * HD                         # 256 local features per core
DC = D // 128                        # 16 contraction chunks
QCW = 512                            # q-chunk width
NQC = S // QCW                       # 4 q-chunks
SCALE = 1.0 / float(np.sqrt(HD))

_CACHED = {}


def _build():
    nc = bacc.Bacc("TRN2", target_bir_lowering=False, debug=False,
                   num_devices=1, enable_asserts=False)

    xt = nc.dram_tensor("xt", [D, S], BF, kind="ExternalInput")        # x^T
    wqt = nc.dram_tensor("wqt", [D, E], BF, kind="ExternalInput")      # q_w shard^T
    wkt = nc.dram_tensor("wkt", [D, E], BF, kind="ExternalInput")
    wvt = nc.dram_tensor("wvt", [D, E], BF, kind="ExternalInput")
    owt = nc.dram_tensor("owt", [E, D], BF, kind="ExternalInput")      # o_w cols^T
    gwt = nc.dram_tensor("gwt", [HD, HD], BF, kind="ExternalInput")    # gate_w^T
    gb = nc.dram_tensor("gb", [HD, 1], F32, kind="ExternalInput")      # gate bias
    trim = nc.dram_tensor("trim", [128, 128], BF, kind="ExternalInput")
    ztrim = nc.dram_tensor("ztrim", [128, 256], BF, kind="ExternalInput")
    yt = nc.dram_tensor("yt", [D, S], BF, kind="ExternalOutput")       # y^T partial

    with tile.TileContext(nc) as tc:
        with tc.tile_pool(name="const", bufs=1) as const, \
             tc.tile_pool(name="work", bufs=2) as work, \
             tc.tile_pool(name="psum", bufs=1, space="PSUM") as psum:

            def pp(name):
                return psum.tile([128, QCW], F32, tag="pp", bufs=4, name=name)

            def pp2(name):
                # two contiguous PSUM banks: 1024 score columns per exp
                return psum.tile([128, 2, QCW], F32, tag="sc2", bufs=2,
                                 name=name)

            # ---- PE warm-up: dummy matmuls on a memset tile release the
            #      HAM clock gate (~3.4us of sustained activity) while the
            #      first input DMAs are still in flight ----
            warm = const.tile([128, QCW], BF, tag="warm", name="warm")
            nc.vector.memset(warm[:], 0.0)
            wp = pp("warmp")
            for _ in range(28):
                nc.tensor.matmul(wp[:], warm[:, 0:128], warm[:],
                                 start=True, stop=True)

            # ---- input loads (few big DMAs; xts chunked to feed the
            #      dc-synchronized projection loop) ----
            wqts = const.tile([128, DC, E], BF, tag="wqts", name="wqts")
            wkts = const.tile([128, DC, E], BF, tag="wkts", name="wkts")
            xts = const.tile([128, DC, S], BF, tag="big", name="xts")

            def _ldw(dst, src, half):
                sl = slice(half * 8, (half + 1) * 8)
                nc.sync.dma_start(
                    dst[:, sl, :],
                    src.ap()[half * 1024:(half + 1) * 1024, :]
                       .rearrange("(c p) e -> p c e", p=128))

            # interleave weight halves with the x chunks so the transfer
            # stream stays just ahead of the first pass's dc-ordered
            # consumption
            def _ldx(d0, d1):
                nc.sync.dma_start(
                    xts[:, d0:d1, :],
                    xt.ap()[d0 * 128:d1 * 128, :]
                      .rearrange("(c p) s -> p c s", p=128))

            _ldw(wqts, wqt, 0)
            _ldx(0, 1)
            _ldw(wkts, wkt, 0)
            for d in range(1, 8):
                _ldx(d, d + 1)
            _ldw(wqts, wqt, 1)
            _ldx(8, 9)
            _ldw(wkts, wkt, 1)
            _ldx(9, 10)
            for k in range(5, 8):
                _ldx(2 * k, 2 * k + 2)

            gwts = const.tile([HD, HD], BF, tag="gwts", name="gwts")
            gbs = const.tile([HD, 1], F32, tag="gbs", name="gbs")
            tris = const.tile([128, 128], BF, tag="tris", name="tris")
            ztris = const.tile([128, 256], BF, tag="ztris", name="ztris")
            ones128 = const.tile([128, 1], BF, tag="ones128", name="ones128")
            nc.sync.dma_start(gwts[:], gwt.ap())
            nc.sync.dma_start(gbs[:], gb.ap())
            nc.sync.dma_start(tris[:], trim.ap())
            nc.sync.dma_start(ztris[:], ztrim.ap())
            nc.vector.memset(ones128[:], 1.0)

            wvts = const.tile([128, DC, E], BF, tag="wvts", name="wvts")
            nc.sync.dma_start(wvts[:], wvt.ap().rearrange("(c p) e -> p c e", p=128))

            # ---- projections ----
            # Q^T, K^T: [e(2x128), s].  Four dc-outer passes of 4 PSUM banks
            # each; pass 1 tracks the streaming xts chunks, later passes
            # re-stream the resident tile.  One weight load per dc per pass.
            qts = const.tile([128, HPC, S], BF, tag="qts", name="qts")
            kts = const.tile([128, HPC, S], BF, tag="kts", name="kts")

            for wts, outts, ec in ((wqts, qts, 0), (wkts, kts, 0),
                                   (wqts, qts, 1), (wkts, kts, 1)):
                pps = [pp("qkp") for _ in range(NQC)]
                for dc in range(DC):
                    st = (dc == 0)
                    sp = (dc == DC - 1)
                    for sc in range(NQC):
                        nc.tensor.matmul(
                            pps[sc][:], wts[:, dc, ec * 128:(ec + 1) * 128],
                            xts[:, dc, sc * QCW:(sc + 1) * QCW],
                            start=st, stop=sp)
                for sc in range(NQC):
                    nc.any.tensor_copy(
                        out=outts[:, ec, sc * QCW:(sc + 1) * QCW],
                        in_=pps[sc][:])

            # o_proj weights [f(2x128), d]: reuse the wqts slot (dead after
            # the loop above)
            owts = const.tile([128, HPC, D], BF, tag="wqts", name="owts")
            nc.sync.dma_start(owts[:], owt.ap().rearrange("(c p) d -> p c d", p=128))

            # gates for both heads, before the V projection so the sigmoid
            # table load and ACT latency hide behind V's matmuls
            gts = const.tile([128, HPC, S], BF, tag="gts", name="gts")
            for h in range(HPC):
                for qc in range(NQC):
                    gp = pp("gp")
                    nc.tensor.matmul(gp[:], gwts[:],
                                     qts[:, h, qc * QCW:(qc + 1) * QCW],
                                     start=True, stop=True)
                    nc.scalar.activation(gts[:, h, qc * QCW:(qc + 1) * QCW],
                                         gp[:], AF.Sigmoid, bias=gbs[:, 0:1])

            # V: [s(16x128), e] natural layout.  Slot-major (xts is fully
            # resident by now): each psum's 16-matmul chain runs while the
            # previous psum's copy drains, so group boundaries don't stall.
            vts = const.tile([128, DC, E], BF, tag="vts", name="vts")
            for sc16 in range(DC):
                vp = pp("vp")
                for dc in range(DC):
                    nc.tensor.matmul(
                        vp[:, :E],
                        xts[:, dc, sc16 * 128:(sc16 + 1) * 128],
                        wvts[:, dc, :], start=(dc == 0), stop=(dc == DC - 1))
                nc.any.tensor_copy(out=vts[:, sc16, :], in_=vp[:, :E])

            # ---- attention (transposed layout), gated output to SBUF ----
            # gavs[f, s]: the per-head gated, normalized attention output,
            # kept on-chip for the partial o_proj.
            gavs = const.tile([128, HPC, S], BF, tag="gavs", name="gavs")

            def emit_oproj(sc):
                # partial o_proj for one seq chunk:
                # y^T[d, s] = sum_{f local} o_w[d, f] gav[f, s]
                for dc in range(DC):
                    yp = pp("yp")
                    for fc in range(HPC):
                        nc.tensor.matmul(
                            yp[:], owts[:, fc, dc * 128:(dc + 1) * 128],
                            gavs[:, fc, sc * QCW:(sc + 1) * QCW],
                            start=(fc == 0), stop=(fc == HPC - 1))
                    ys = work.tile([128, QCW], BF, tag="ys", bufs=6,
                                   name="ys")
                    nc.any.tensor_copy(out=ys[:], in_=yp[:])
                    nc.sync.dma_start(
                        yt.ap()[dc * 128:(dc + 1) * 128,
                                sc * QCW:(sc + 1) * QCW],
                        ys[:])

            # Software-pipelined across (h, qc) blocks at key-pair (group)
            # granularity: each block's last AV pair and its epilogue are
            # emitted after the NEXT block's first scores/exp, so the PE
            # never idles waiting for the tail exp on ACT.
            pend = None   # deferred tail of the previous block

            def emit_tail_av(t):
                # deferred AV matmuls for the last group's two chunks
                (h, q0, avp, gacc, ext2_l, s0s, njj) = t
                for c in (0, 1):
                    jj = njj - 2 + c
                    nc.tensor.matmul(
                        avp[:, s0s[c]:], vts[:, jj, h * 128:(h + 1) * 128],
                        ext2_l[:, c, s0s[c]:], start=False, stop=(c == 1))

            def emit_tail(t):
                (h, q0, avp, gacc, ext2_l, s0s, njj) = t
                # single ones-matmul folds the DVE-accumulated exp sums
                # across partitions (row sums of the softmax numerator)
                sump = psum.tile([1, QCW], F32, tag="pp", bufs=4, name="sump")
                nc.tensor.matmul(sump[:], ones128[:], gacc[:],
                                 start=True, stop=True)
                rs = work.tile([1, QCW], F32, tag="rs", bufs=2, name="rs")
                nc.vector.reciprocal(out=rs[:], in_=sump[:])
                # broadcast 1/sum across partitions on the (idle) Pool engine
                # so the epilogue never blocks the PE
                bcb = work.tile([128, QCW], F32, tag="bcb", bufs=2, name="bcb")
                nc.gpsimd.partition_broadcast(bcb[:], rs[:])
                gn = work.tile([128, QCW], BF, tag="gn", bufs=2, name="gn")
                nc.any.tensor_mul(gn[:], gts[:, h, q0:q0 + QCW], bcb[:])
                nc.any.tensor_mul(gavs[:, h, q0:q0 + QCW], avp[:], gn[:])

            for h in range(HPC):
                for qc in range(NQC):
                    q0 = qc * QCW
                    avp = pp("avp")
                    gacc = work.tile([128, QCW], BF, tag="gacc", bufs=2,
                                     name="gacc")
                    njj = 4 * qc + 4
                    G = njj // 2
                    ext2s = [None, None, None]

                    def s0_of(jj):
                        return max(0, (jj - 4 * qc) * 128)

                    def emit_av(g):
                        for c in (0, 1):
                            jj = 2 * g + c
                            s0 = s0_of(jj)
                            nc.tensor.matmul(
                                avp[:, s0:],
                                vts[:, jj, h * 128:(h + 1) * 128],
                                ext2s[g % 3][:, c, s0:],
                                start=(jj == 0), stop=False)

                    for g in range(G):
                        jj0 = 2 * g
                        off0 = jj0 - 4 * qc
                        s00 = s0_of(jj0)
                        sc2 = pp2("sc2")
                        ext2 = work.tile([128, 2, QCW], BF, tag="ext",
                                         bufs=3, name="ext2")
                        ext2s[g % 3] = ext2
                        # both chunks computed from s00 so one wide exp
                        # covers the pair; the sub-diagonal sliver of the
                        # second chunk is zeroed by the mask
                        nc.tensor.matmul(
                            sc2[:, 0, s00:],
                            kts[:, h, jj0 * 128:(jj0 + 1) * 128],
                            qts[:, h, q0 + s00:q0 + QCW],
                            start=True, stop=True)
                        nc.tensor.matmul(
                            sc2[:, 1, s00:],
                            kts[:, h, (jj0 + 1) * 128:(jj0 + 2) * 128],
                            qts[:, h, q0 + s00:q0 + QCW],
                            start=True, stop=True)
                        nc.scalar.activation(ext2[:, :, s00:],
                                             sc2[:, :, s00:],
                                             AF.Exp, scale=SCALE)
                        if off0 >= 0:
                            # diagonal pair: chunk0 keeps q >= its diag,
                            # chunk1 additionally zeroes the 128 columns
                            # below its own diagonal
                            nc.vector.tensor_mul(
                                ext2[:, 0, s00:s00 + 128],
                                ext2[:, 0, s00:s00 + 128], tris[:])
                            nc.vector.tensor_mul(
                                ext2[:, 1, s00:s00 + 256],
                                ext2[:, 1, s00:s00 + 256], ztris[:])
                        # running exp row-sum accumulates on the DVE/ACT so
                        # the PE only pays one ones-matmul per block
                        if g == 0:
                            nc.any.tensor_copy(out=gacc[:], in_=ext2[:, 0, :])
                        else:
                            nc.any.tensor_add(gacc[:, s00:], gacc[:, s00:],
                                              ext2[:, 0, s00:])
                        s01 = s0_of(jj0 + 1)
                        nc.any.tensor_add(gacc[:, s01:], gacc[:, s01:],
                                          ext2[:, 1, s01:])
                        if pend is not None:
                            emit_tail_av(pend)
                            emit_tail(pend)
                            pend = None
                        if g >= 1:
                            emit_av(g - 1)
                    cur = (h, q0, avp, gacc, ext2s[(G - 1) % 3],
                           (s0_of(njj - 2), s0_of(njj - 1)), njj)
                    if h == 1:
                        # second head: finish the block's tail immediately,
                        # then interleave this seq chunk's o_proj — its
                        # matmuls fill the PE slack of the ACT-paced softmax
                        # and spread the output DMA across the whole phase
                        emit_tail_av(cur)
                        emit_tail(cur)
                        emit_oproj(qc)
                    else:
                        pend = cur
                if h == 0:
                    # flush at the head boundary
                    emit_tail_av(pend)
                    emit_tail(pend)
                    pend = None

    nc.compile()
    return nc


def _prep_inputs(x, q_w, k_w, v_w, o_w, gate_w, gate_b):
    x = np.asarray(x, dtype=np.float32)
    xt = np.ascontiguousarray(x.reshape(S, D).T).astype(BF16)
    gwt = np.ascontiguousarray(np.asarray(gate_w, np.float32).T).astype(BF16)
    gb = np.asarray(gate_b, np.float32).reshape(HD, 1).copy()
    trim = np.triu(np.ones((128, 128), np.float32)).astype(BF16)
    ztrim = np.hstack([np.zeros((128, 128), np.float32),
                       np.triu(np.ones((128, 128), np.float32))]).astype(BF16)
    ow = np.asarray(o_w, np.float32)
    in_maps = []
    for c in range(N_CORES):
        sl = slice(c * E, (c + 1) * E)
        in_maps.append({
            "xt": xt,
            "wqt": np.ascontiguousarray(np.asarray(q_w, np.float32)[sl, :].T).astype(BF16),
            "wkt": np.ascontiguousarray(np.asarray(k_w, np.float32)[sl, :].T).astype(BF16),
            "wvt": np.ascontiguousarray(np.asarray(v_w, np.float32)[sl, :].T).astype(BF16),
            "owt": np.ascontiguousarray(ow[:, sl].T).astype(BF16),
            "gwt": gwt,
            "gb": gb,
            "trim": trim,
            "ztrim": ztrim,
        })
    return in_maps


def _run(in_maps, **kwargs):
    if "nc" not in _CACHED:
        _CACHED["nc"] = _build()
    return run_bass_kernel_spmd(_CACHED["nc"], in_maps,
                                core_ids=list(range(N_CORES)), **kwargs)


def kernel(x, q_w, k_w, v_w, o_w, gate_w, gate_b):
    res = _run(_prep_inputs(x, q_w, k_w, v_w, o_w, gate_w, gate_b))
    y_t = res.results[0]["yt"].astype(np.float32)
    for c in range(1, N_CORES):
        y_t += res.results[c]["yt"].astype(np.float32)
    return np.ascontiguousarray(y_t.T, dtype=np.float32).reshape(1, S, D)


# revision 14
# speedup vs baseline: 1.0530x; 1.0530x over previous
"""GatedAttention Trainium2 kernel, 8-way tensor-parallel over heads.

Reference computation (B=1, S=2048, D=2048, H=16 heads, Hd=128):
  q,k,v = x @ {q,k,v}_w.T  (per-head split)
  scores = (q @ k.T) / sqrt(Hd), causal mask, softmax
  av = attn @ v
  gate = sigmoid(q @ gate_w.T + gate_b)       (per-head)
  y = concat_heads(av * gate) @ o_w.T

Sharding: 2 heads per core (column-parallel QKV/gate). o_proj is computed
as a per-core PARTIAL product over the core's 256 local features into the
full [D, S] output; the host sums the 8 partials. No collectives at all —
each core's program is fully independent, so no core ever waits on
another.

All matmuls run on the PE in bf16 with fp32 PSUM accumulation. Softmax runs
without max-subtraction (scores are small by construction). Attention works
in the transposed [key, query] layout so no on-chip transposes are needed;
exps run 1024 columns at a time (two key chunks share one two-bank PSUM
tile) to halve the ACT instruction count, and exp row-sums accumulate on
the DVE so the PE only pays one ones-matmul per query block. o_proj chunks
are interleaved into the second head's attention stream, filling the PE
slack left by the ACT-paced softmax. A short burst of dummy matmuls at t=0
warms the PE clock gate (HAM) while the first input DMAs are in flight.
"""

import numpy as np
import ml_dtypes

import concourse.bass as bass
import concourse.mybir as mybir
import concourse.tile as tile
from concourse import bacc
from concourse.bass_utils import run_bass_kernel_spmd

BF16 = ml_dtypes.bfloat16
F32 = mybir.dt.float32
BF = mybir.dt.bfloat16
AF = mybir.ActivationFunctionType

N_CORES = 8
S = 2048          # sequence length
D = 2048          # model dim
H = 16            # total heads
HD = 128          # head dim
HPC = H // N_CORES                   # heads per core: 2
E = HPC * HD                         # 256 local features per core
DC = D // 128                        # 16 contraction chunks
QCW = 512                            # q-chunk width
NQC = S // QCW                       # 4 q-chunks
SCALE = 1.0 / float(np.sqrt(HD))

_CACHED = {}


def _build():
    nc = bacc.Bacc("TRN2", target_bir_lowering=False, debug=False,
                   num_devices=1, enable_asserts=False)

    xt = nc.dram_tensor("xt", [D, S], BF, kind="ExternalInput")        # x^T
    wqt = nc.dram_tensor("wqt", [D, E], BF, kind="ExternalInput")      # q_w shard^T
    wkt = nc.dram_tensor("wkt", [D, E], BF, kind="ExternalInput")
    wvt = nc.dram_tensor("wvt", [D, E], BF, kind="ExternalInput")
    owt = nc.dram_tensor("owt", [E, D], BF, kind="ExternalInput")      # o_w cols^T
    gwt = nc.dram_tensor("gwt", [HD, HD], BF, kind="ExternalInput")    # gate_w^T
    gb = nc.dram_tensor("gb", [HD, 1], F32, kind="ExternalInput")      # gate bias
    trim = nc.dram_tensor("trim", [128, 128], BF, kind="ExternalInput")
    ztrim = nc.dram_tensor("ztrim", [128, 256], BF, kind="ExternalInput")
    yt = nc.dram_tensor("yt", [D, S], BF, kind="ExternalOutput")       # y^T partial

    with tile.TileContext(nc) as tc:
        with tc.tile_pool(name="const", bufs=1) as const, \
             tc.tile_pool(name="work", bufs=2) as work, \
             tc.tile_pool(name="psum", bufs=1, space="PSUM") as psum:

            def pp(name):
                return psum.tile([128, QCW], F32, tag="pp", bufs=4, name=name)

            def pp2(name):
                # two contiguous PSUM banks: 1024 score columns per exp
                return psum.tile([128, 2, QCW], F32, tag="sc2", bufs=2,
                                 name=name)

            # ---- PE warm-up: dummy matmuls on a memset tile release the
            #      HAM clock gate (~3.4us of sustained activity) while the
            #      first input DMAs are still in flight ----
            warm = const.tile([128, QCW], BF, tag="warm", name="warm")
            nc.vector.memset(warm[:], 0.0)
            wp = pp("warmp")
            for _ in range(28):
                nc.tensor.matmul(wp[:], warm[:, 0:128], warm[:],
                                 start=True, stop=True)

            # ---- input loads (few big DMAs; xts chunked to feed the
            #      dc-synchronized projection loop) ----
            wqts = const.tile([128, DC, E], BF, tag="wqts", name="wqts")
            wkts = const.tile([128, DC, E], BF, tag="wkts", name="wkts")
            xts = const.tile([128, DC, S], BF, tag="big", name="xts")

            def _ldw(dst, src, half):
                sl = slice(half * 8, (half + 1) * 8)
                nc.sync.dma_start(
                    dst[:, sl, :],
                    src.ap()[half * 1024:(half + 1) * 1024, :]
                       .rearrange("(c p) e -> p c e", p=128))

            # interleave weight halves with the x chunks so the transfer
            # stream stays just ahead of the first pass's dc-ordered
            # consumption
            def _ldx(d0, d1):
                nc.sync.dma_start(
                    xts[:, d0:d1, :],
                    xt.ap()[d0 * 128:d1 * 128, :]
                      .rearrange("(c p) s -> p c s", p=128))

            _ldw(wqts, wqt, 0)
            _ldx(0, 1)
            _ldw(wkts, wkt, 0)
            for d in range(1, 8):
                _ldx(d, d + 1)
            _ldw(wqts, wqt, 1)
            _ldx(8, 9)
            _ldw(wkts, wkt, 1)
            _ldx(9, 10)
            for k in range(5, 8):
                _ldx(2 * k, 2 * k + 2)

            gwts = const.tile([HD, HD], BF, tag="gwts", name="gwts")
            gbs = const.tile([HD, 1], F32, tag="gbs", name="gbs")
            tris = const.tile([128, 128], BF, tag="tris", name="tris")
            ztris = const.tile([128, 256], BF, tag="ztris", name="ztris")
            ones128 = const.tile([128, 1], BF, tag="ones128", name="ones128")
            nc.sync.dma_start(gwts[:], gwt.ap())
            nc.sync.dma_start(gbs[:], gb.ap())
            nc.sync.dma_start(tris[:], trim.ap())
            nc.sync.dma_start(ztris[:], ztrim.ap())
            nc.vector.memset(ones128[:], 1.0)

            wvts = const.tile([128, DC, E], BF, tag="wvts", name="wvts")
            nc.sync.dma_start(wvts[:], wvt.ap().rearrange("(c p) e -> p c e", p=128))

            # ---- projections ----
            # Q^T, K^T: [e(2x128), s].  Four dc-outer passes of 4 PSUM banks
            # each; pass 1 tracks the streaming xts chunks, later passes
            # re-stream the resident tile.  One weight load per dc per pass.
            qts = const.tile([128, HPC, S], BF, tag="qts", name="qts")
            kts = const.tile([128, HPC, S], BF, tag="kts", name="kts")

            # two dc-outer passes (one per e-chunk); Q accumulates in the 4
            # plain banks, K in the two 2-bank sc2 tiles (idle until
            # attention), so each pass keeps 8 matmuls per dc in flight —
            # enough PE work to track the streaming xts chunks
            for ec in range(HPC):
                qaccs = [pp("qkp") for _ in range(NQC)]
                kaccs = [pp2("ksc") for _ in range(2)]
                for dc in range(DC):
                    st = (dc == 0)
                    sp = (dc == DC - 1)
                    for sc in range(NQC):
                        nc.tensor.matmul(
                            qaccs[sc][:], wqts[:, dc, ec * 128:(ec + 1) * 128],
                            xts[:, dc, sc * QCW:(sc + 1) * QCW],
                            start=st, stop=sp)
                    for sc in range(NQC):
                        nc.tensor.matmul(
                            kaccs[sc // 2][:, sc % 2, :],
                            wkts[:, dc, ec * 128:(ec + 1) * 128],
                            xts[:, dc, sc * QCW:(sc + 1) * QCW],
                            start=st, stop=sp)
                for sc in range(NQC):
                    nc.any.tensor_copy(
                        out=qts[:, ec, sc * QCW:(sc + 1) * QCW],
                        in_=qaccs[sc][:])
                    nc.any.tensor_copy(
                        out=kts[:, ec, sc * QCW:(sc + 1) * QCW],
                        in_=kaccs[sc // 2][:, sc % 2, :])

            # o_proj weights [f(2x128), d]: reuse the wqts slot (dead after
            # the loop above)
            owts = const.tile([128, HPC, D], BF, tag="wqts", name="owts")
            nc.sync.dma_start(owts[:], owt.ap().rearrange("(c p) d -> p c d", p=128))

            # gates for both heads, before the V projection so the sigmoid
            # table load and ACT latency hide behind V's matmuls
            gts = const.tile([128, HPC, S], BF, tag="gts", name="gts")
            for h in range(HPC):
                for qc in range(NQC):
                    gp = pp("gp")
                    nc.tensor.matmul(gp[:], gwts[:],
                                     qts[:, h, qc * QCW:(qc + 1) * QCW],
                                     start=True, stop=True)
                    nc.scalar.activation(gts[:, h, qc * QCW:(qc + 1) * QCW],
                                         gp[:], AF.Sigmoid, bias=gbs[:, 0:1])

            # V: [s(16x128), e] natural layout.  Slot-major (xts is fully
            # resident by now): each psum's 16-matmul chain runs while the
            # previous psum's copy drains, so group boundaries don't stall.
            vts = const.tile([128, DC, E], BF, tag="vts", name="vts")
            for sc16 in range(DC):
                vp = pp("vp")
                for dc in range(DC):
                    nc.tensor.matmul(
                        vp[:, :E],
                        xts[:, dc, sc16 * 128:(sc16 + 1) * 128],
                        wvts[:, dc, :], start=(dc == 0), stop=(dc == DC - 1))
                nc.any.tensor_copy(out=vts[:, sc16, :], in_=vp[:, :E])

            # ---- attention (transposed layout), gated output to SBUF ----
            # gavs[f, s]: the per-head gated, normalized attention output,
            # kept on-chip for the partial o_proj.
            gavs = const.tile([128, HPC, S], BF, tag="gavs", name="gavs")

            def emit_oproj(sc):
                # partial o_proj for one seq chunk:
                # y^T[d, s] = sum_{f local} o_w[d, f] gav[f, s]
                for dc in range(DC):
                    yp = pp("yp")
                    for fc in range(HPC):
                        nc.tensor.matmul(
                            yp[:], owts[:, fc, dc * 128:(dc + 1) * 128],
                            gavs[:, fc, sc * QCW:(sc + 1) * QCW],
                            start=(fc == 0), stop=(fc == HPC - 1))
                    ys = work.tile([128, QCW], BF, tag="ys", bufs=6,
                                   name="ys")
                    nc.any.tensor_copy(out=ys[:], in_=yp[:])
                    nc.sync.dma_start(
                        yt.ap()[dc * 128:(dc + 1) * 128,
                                sc * QCW:(sc + 1) * QCW],
                        ys[:])

            # Software-pipelined across (h, qc) blocks at key-pair (group)
            # granularity: each block's last AV pair and its epilogue are
            # emitted after the NEXT block's first scores/exp, so the PE
            # never idles waiting for the tail exp on ACT.
            pend = None   # deferred tail of the previous block

            def emit_tail_av(t):
                # deferred AV matmuls for the last group's two chunks
                (h, q0, avp, gacc, ext2_l, s0s, njj) = t
                for c in (0, 1):
                    jj = njj - 2 + c
                    nc.tensor.matmul(
                        avp[:, s0s[c]:], vts[:, jj, h * 128:(h + 1) * 128],
                        ext2_l[:, c, s0s[c]:], start=False, stop=(c == 1))

            def emit_tail(t):
                (h, q0, avp, gacc, ext2_l, s0s, njj) = t
                # single ones-matmul folds the DVE-accumulated exp sums
                # across partitions (row sums of the softmax numerator)
                sump = psum.tile([1, QCW], F32, tag="pp", bufs=4, name="sump")
                nc.tensor.matmul(sump[:], ones128[:], gacc[:],
                                 start=True, stop=True)
                rs = work.tile([1, QCW], F32, tag="rs", bufs=2, name="rs")
                nc.vector.reciprocal(out=rs[:], in_=sump[:])
                # broadcast 1/sum across partitions on the (idle) Pool engine
                # so the epilogue never blocks the PE
                bcb = work.tile([128, QCW], F32, tag="bcb", bufs=2, name="bcb")
                nc.gpsimd.partition_broadcast(bcb[:], rs[:])
                gn = work.tile([128, QCW], BF, tag="gn", bufs=2, name="gn")
                nc.any.tensor_mul(gn[:], gts[:, h, q0:q0 + QCW], bcb[:])
                nc.any.tensor_mul(gavs[:, h, q0:q0 + QCW], avp[:], gn[:])

            for h in range(HPC):
                for qc in range(NQC):
                    q0 = qc * QCW
                    avp = pp("avp")
                    gacc = work.tile([128, QCW], BF, tag="gacc", bufs=2,
                                     name="gacc")
                    njj = 4 * qc + 4
                    G = njj // 2
                    ext2s = [None, None, None]

                    def s0_of(jj):
                        return max(0, (jj - 4 * qc) * 128)

                    def emit_av(g):
                        for c in (0, 1):
                            jj = 2 * g + c
                            s0 = s0_of(jj)
                            nc.tensor.matmul(
                                avp[:, s0:],
                                vts[:, jj, h * 128:(h + 1) * 128],
                                ext2s[g % 3][:, c, s0:],
                                start=(jj == 0), stop=False)

                    for g in range(G):
                        jj0 = 2 * g
                        off0 = jj0 - 4 * qc
                        s00 = s0_of(jj0)
                        sc2 = pp2("sc2")
                        ext2 = work.tile([128, 2, QCW], BF, tag="ext",
                                         bufs=3, name="ext2")
                        ext2s[g % 3] = ext2
                        # both chunks computed from s00 so one wide exp
                        # covers the pair; the sub-diagonal sliver of the
                        # second chunk is zeroed by the mask
                        nc.tensor.matmul(
                            sc2[:, 0, s00:],
                            kts[:, h, jj0 * 128:(jj0 + 1) * 128],
                            qts[:, h, q0 + s00:q0 + QCW],
                            start=True, stop=True)
                        nc.tensor.matmul(
                            sc2[:, 1, s00:],
                            kts[:, h, (jj0 + 1) * 128:(jj0 + 2) * 128],
                            qts[:, h, q0 + s00:q0 + QCW],
                            start=True, stop=True)
                        nc.scalar.activation(ext2[:, :, s00:],
                                             sc2[:, :, s00:],
                                             AF.Exp, scale=SCALE)
                        if off0 >= 0:
                            # diagonal pair: chunk0 keeps q >= its diag,
                            # chunk1 additionally zeroes the 128 columns
                            # below its own diagonal
                            nc.vector.tensor_mul(
                                ext2[:, 0, s00:s00 + 128],
                                ext2[:, 0, s00:s00 + 128], tris[:])
                            nc.vector.tensor_mul(
                                ext2[:, 1, s00:s00 + 256],
                                ext2[:, 1, s00:s00 + 256], ztris[:])
                        # running exp row-sum accumulates on the DVE/ACT so
                        # the PE only pays one ones-matmul per block
                        if g == 0:
                            nc.any.tensor_copy(out=gacc[:], in_=ext2[:, 0, :])
                        else:
                            nc.any.tensor_add(gacc[:, s00:], gacc[:, s00:],
                                              ext2[:, 0, s00:])
                        s01 = s0_of(jj0 + 1)
                        nc.any.tensor_add(gacc[:, s01:], gacc[:, s01:],
                                          ext2[:, 1, s01:])
                        if pend is not None:
                            emit_tail_av(pend)
                            emit_tail(pend)
                            pend = None
                        if g >= 1:
                            emit_av(g - 1)
                    cur = (h, q0, avp, gacc, ext2s[(G - 1) % 3],
                           (s0_of(njj - 2), s0_of(njj - 1)), njj)
                    if h == 1:
                        # second head: finish the block's tail immediately,
                        # then interleave this seq chunk's o_proj — its
                        # matmuls fill the PE slack of the ACT-paced softmax
                        # and spread the output DMA across the whole phase
                        emit_tail_av(cur)
                        emit_tail(cur)
                        emit_oproj(qc)
                    else:
                        pend = cur
                if h == 0:
                    # flush at the head boundary
                    emit_tail_av(pend)
                    emit_tail(pend)
                    pend = None

    nc.compile()
    return nc


def _prep_inputs(x, q_w, k_w, v_w, o_w, gate_w, gate_b):
    x = np.asarray(x, dtype=np.float32)
    xt = np.ascontiguousarray(x.reshape(S, D).T).astype(BF16)
    gwt = np.ascontiguousarray(np.asarray(gate_w, np.float32).T).astype(BF16)
    gb = np.asarray(gate_b, np.float32).reshape(HD, 1).copy()
    trim = np.triu(np.ones((128, 128), np.float32)).astype(BF16)
    ztrim = np.hstack([np.zeros((128, 128), np.float32),
                       np.triu(np.ones((128, 128), np.float32))]).astype(BF16)
    ow = np.asarray(o_w, np.float32)
    in_maps = []
    for c in range(N_CORES):
        sl = slice(c * E, (c + 1) * E)
        in_maps.append({
            "xt": xt,
            "wqt": np.ascontiguousarray(np.asarray(q_w, np.float32)[sl, :].T).astype(BF16),
            "wkt": np.ascontiguousarray(np.asarray(k_w, np.float32)[sl, :].T).astype(BF16),
            "wvt": np.ascontiguousarray(np.asarray(v_w, np.float32)[sl, :].T).astype(BF16),
            "owt": np.ascontiguousarray(ow[:, sl].T).astype(BF16),
            "gwt": gwt,
            "gb": gb,
            "trim": trim,
            "ztrim": ztrim,
        })
    return in_maps


def _run(in_maps, **kwargs):
    if "nc" not in _CACHED:
        _CACHED["nc"] = _build()
    return run_bass_kernel_spmd(_CACHED["nc"], in_maps,
                                core_ids=list(range(N_CORES)), **kwargs)


def kernel(x, q_w, k_w, v_w, o_w, gate_w, gate_b):
    res = _run(_prep_inputs(x, q_w, k_w, v_w, o_w, gate_w, gate_b))
    y_t = res.results[0]["yt"].astype(np.float32)
    for c in range(1, N_CORES):
        y_t += res.results[c]["yt"].astype(np.float32)
    return np.ascontiguousarray(y_t.T, dtype=np.float32).reshape(1, S, D)


# revision 19
# speedup vs baseline: 1.0593x; 1.0060x over previous
"""GatedAttention Trainium2 kernel, 8-way tensor-parallel over heads.

Reference computation (B=1, S=2048, D=2048, H=16 heads, Hd=128):
  q,k,v = x @ {q,k,v}_w.T  (per-head split)
  scores = (q @ k.T) / sqrt(Hd), causal mask, softmax
  av = attn @ v
  gate = sigmoid(q @ gate_w.T + gate_b)       (per-head)
  y = concat_heads(av * gate) @ o_w.T

Sharding: 2 heads per core (column-parallel QKV/gate). o_proj is computed
as a per-core PARTIAL product over the core's 256 local features into the
full [D, S] output; the host sums the 8 partials. No collectives at all —
each core's program is fully independent, so no core ever waits on
another.

All matmuls run on the PE in bf16 with fp32 PSUM accumulation. Softmax runs
without max-subtraction (scores are small by construction). Attention works
in the transposed [key, query] layout so no on-chip transposes are needed;
exps run 1024 columns at a time (two key chunks share one two-bank PSUM
tile) to halve the ACT instruction count, and exp row-sums accumulate on
the DVE so the PE only pays one ones-matmul per query block. o_proj chunks
are interleaved into the second head's attention stream, filling the PE
slack left by the ACT-paced softmax. A short burst of dummy matmuls at t=0
warms the PE clock gate (HAM) while the first input DMAs are in flight.
"""

import numpy as np
import ml_dtypes

import concourse.bass as bass
import concourse.mybir as mybir
import concourse.tile as tile
from concourse import bacc
from concourse.bass_utils import run_bass_kernel_spmd

BF16 = ml_dtypes.bfloat16
F32 = mybir.dt.float32
BF = mybir.dt.bfloat16
AF = mybir.ActivationFunctionType

N_CORES = 8
S = 2048          # sequence length
D = 2048          # model dim
H = 16            # total heads
HD = 128          # head dim
HPC = H // N_CORES                   # heads per core: 2
E = HPC * HD                         # 256 local features per core
DC = D // 128                        # 16 contraction chunks
QCW = 512                            # q-chunk width
NQC = S // QCW                       # 4 q-chunks
SCALE = 1.0 / float(np.sqrt(HD))

_CACHED = {}


def _build():
    nc = bacc.Bacc("TRN2", target_bir_lowering=False, debug=False,
                   num_devices=1, enable_asserts=False)

    xt = nc.dram_tensor("xt", [D, S], BF, kind="ExternalInput")        # x^T
    wqt = nc.dram_tensor("wqt", [D, E], BF, kind="ExternalInput")      # q_w shard^T
    wkt = nc.dram_tensor("wkt", [D, E], BF, kind="ExternalInput")
    wvt = nc.dram_tensor("wvt", [D, E], BF, kind="ExternalInput")
    owt = nc.dram_tensor("owt", [E, D], BF, kind="ExternalInput")      # o_w cols^T
    gwt = nc.dram_tensor("gwt", [HD, HD], BF, kind="ExternalInput")    # gate_w^T
    gb = nc.dram_tensor("gb", [HD, 1], F32, kind="ExternalInput")      # gate bias
    trim = nc.dram_tensor("trim", [128, 128], BF, kind="ExternalInput")
    ztrim = nc.dram_tensor("ztrim", [128, 256], BF, kind="ExternalInput")
    yt = nc.dram_tensor("yt", [D, S], BF, kind="ExternalOutput")       # y^T partial

    with tile.TileContext(nc) as tc:
        with tc.tile_pool(name="const", bufs=1) as const, \
             tc.tile_pool(name="work", bufs=2) as work, \
             tc.tile_pool(name="psum", bufs=1, space="PSUM") as psum:

            def pp(name):
                return psum.tile([128, QCW], F32, tag="pp", bufs=4, name=name)

            def pp2(name):
                # two contiguous PSUM banks: 1024 score columns per exp
                return psum.tile([128, 2, QCW], F32, tag="sc2", bufs=2,
                                 name=name)

            # ---- PE warm-up: dummy matmuls on a memset tile release the
            #      HAM clock gate (~3.4us of sustained activity) while the
            #      first input DMAs are still in flight ----
            warm = const.tile([128, QCW], BF, tag="warm", name="warm")
            nc.vector.memset(warm[:], 0.0)
            wp = pp("warmp")
            for _ in range(28):
                nc.tensor.matmul(wp[:], warm[:, 0:128], warm[:],
                                 start=True, stop=True)

            # ---- input loads (few big DMAs; xts chunked to feed the
            #      dc-synchronized projection loop) ----
            wqts = const.tile([128, DC, E], BF, tag="wqts", name="wqts")
            wkts = const.tile([128, DC, E], BF, tag="wkts", name="wkts")
            xts = const.tile([128, DC, S], BF, tag="big", name="xts")

            def _ldw(dst, src, half):
                sl = slice(half * 8, (half + 1) * 8)
                nc.sync.dma_start(
                    dst[:, sl, :],
                    src.ap()[half * 1024:(half + 1) * 1024, :]
                       .rearrange("(c p) e -> p c e", p=128))

            # interleave weight halves with the x chunks so the transfer
            # stream stays just ahead of the first pass's dc-ordered
            # consumption
            def _ldx(d0, d1):
                nc.sync.dma_start(
                    xts[:, d0:d1, :],
                    xt.ap()[d0 * 128:d1 * 128, :]
                      .rearrange("(c p) s -> p c s", p=128))

            _ldw(wqts, wqt, 0)
            _ldx(0, 1)
            _ldw(wkts, wkt, 0)
            for d in range(1, 8):
                _ldx(d, d + 1)
            _ldw(wqts, wqt, 1)
            _ldx(8, 9)
            _ldw(wkts, wkt, 1)
            _ldx(9, 10)
            for k in range(5, 8):
                _ldx(2 * k, 2 * k + 2)

            gwts = const.tile([HD, HD], BF, tag="gwts", name="gwts")
            gbs = const.tile([HD, 1], F32, tag="gbs", name="gbs")
            tris = const.tile([128, 128], BF, tag="tris", name="tris")
            ztris = const.tile([128, 256], BF, tag="ztris", name="ztris")
            ones128 = const.tile([128, 1], BF, tag="ones128", name="ones128")
            ones1 = const.tile([1, 128], BF, tag="ones1", name="ones1")
            nc.vector.memset(ones1[:], 1.0)
            nc.sync.dma_start(gwts[:], gwt.ap())
            nc.sync.dma_start(gbs[:], gb.ap())
            nc.sync.dma_start(tris[:], trim.ap())
            nc.sync.dma_start(ztris[:], ztrim.ap())
            nc.vector.memset(ones128[:], 1.0)

            wvts = const.tile([128, DC, E], BF, tag="wvts", name="wvts")
            nc.sync.dma_start(wvts[:], wvt.ap().rearrange("(c p) e -> p c e", p=128))

            # ---- projections ----
            # Q^T, K^T: [e(2x128), s].  Four dc-outer passes of 4 PSUM banks
            # each; pass 1 tracks the streaming xts chunks, later passes
            # re-stream the resident tile.  One weight load per dc per pass.
            qts = const.tile([128, HPC, S], BF, tag="qts", name="qts")
            kts = const.tile([128, HPC, S], BF, tag="kts", name="kts")

            # two dc-outer passes (one per e-chunk); Q accumulates in the 4
            # plain banks, K in the two 2-bank sc2 tiles (idle until
            # attention), so each pass keeps 8 matmuls per dc in flight —
            # enough PE work to track the streaming xts chunks
            for ec in range(HPC):
                qaccs = [pp("qkp") for _ in range(NQC)]
                kaccs = [pp2("ksc") for _ in range(2)]
                for dc in range(DC):
                    st = (dc == 0)
                    sp = (dc == DC - 1)
                    for sc in range(NQC):
                        nc.tensor.matmul(
                            qaccs[sc][:], wqts[:, dc, ec * 128:(ec + 1) * 128],
                            xts[:, dc, sc * QCW:(sc + 1) * QCW],
                            start=st, stop=sp)
                    for sc in range(NQC):
                        nc.tensor.matmul(
                            kaccs[sc // 2][:, sc % 2, :],
                            wkts[:, dc, ec * 128:(ec + 1) * 128],
                            xts[:, dc, sc * QCW:(sc + 1) * QCW],
                            start=st, stop=sp)
                for sc in range(NQC):
                    nc.any.tensor_copy(
                        out=qts[:, ec, sc * QCW:(sc + 1) * QCW],
                        in_=qaccs[sc][:])
                    nc.any.tensor_copy(
                        out=kts[:, ec, sc * QCW:(sc + 1) * QCW],
                        in_=kaccs[sc // 2][:, sc % 2, :])

            # o_proj weights [f(2x128), d]: reuse the wqts slot (dead after
            # the loop above)
            owts = const.tile([128, HPC, D], BF, tag="wqts", name="owts")
            nc.sync.dma_start(owts[:], owt.ap().rearrange("(c p) d -> p c d", p=128))

            # gates for both heads, before the V projection so the sigmoid
            # table load and ACT latency hide behind V's matmuls
            gts = const.tile([128, HPC, S], BF, tag="gts", name="gts")
            for h in range(HPC):
                for qc in range(NQC):
                    gp = pp("gp")
                    nc.tensor.matmul(gp[:], gwts[:],
                                     qts[:, h, qc * QCW:(qc + 1) * QCW],
                                     start=True, stop=True)
                    nc.scalar.activation(gts[:, h, qc * QCW:(qc + 1) * QCW],
                                         gp[:], AF.Sigmoid, bias=gbs[:, 0:1])

            # V: [s(16x128), e] natural layout.  Slot-major (xts is fully
            # resident by now): each psum's 16-matmul chain runs while the
            # previous psum's copy drains, so group boundaries don't stall.
            vts = const.tile([128, DC, E], BF, tag="vts", name="vts")
            for sc16 in range(DC):
                vp = pp("vp")
                for dc in range(DC):
                    nc.tensor.matmul(
                        vp[:, :E],
                        xts[:, dc, sc16 * 128:(sc16 + 1) * 128],
                        wvts[:, dc, :], start=(dc == 0), stop=(dc == DC - 1))
                nc.any.tensor_copy(out=vts[:, sc16, :], in_=vp[:, :E])

            # ---- attention (transposed layout), gated output to SBUF ----
            # gavs[f, s]: the per-head gated, normalized attention output,
            # kept on-chip for the partial o_proj.
            gavs = const.tile([128, HPC, S], BF, tag="gavs", name="gavs")

            def emit_oproj(sc):
                # partial o_proj for one seq chunk:
                # y^T[d, s] = sum_{f local} o_w[d, f] gav[f, s]
                for dc in range(DC):
                    yp = pp("yp")
                    for fc in range(HPC):
                        nc.tensor.matmul(
                            yp[:], owts[:, fc, dc * 128:(dc + 1) * 128],
                            gavs[:, fc, sc * QCW:(sc + 1) * QCW],
                            start=(fc == 0), stop=(fc == HPC - 1))
                    ys = work.tile([128, QCW], BF, tag="ys", bufs=6,
                                   name="ys")
                    nc.any.tensor_copy(out=ys[:], in_=yp[:])
                    nc.sync.dma_start(
                        yt.ap()[dc * 128:(dc + 1) * 128,
                                sc * QCW:(sc + 1) * QCW],
                        ys[:])

            # Software-pipelined across (h, qc) blocks at key-pair (group)
            # granularity: each block's last AV pair and its epilogue are
            # emitted after the NEXT block's first scores/exp, so the PE
            # never idles waiting for the tail exp on ACT.
            pend = None   # deferred tail of the previous block

            def emit_tail_av(t):
                # deferred AV matmuls for the last group's two chunks
                (h, q0, avp, gacc, ext2_l, s0s, njj) = t
                for c in (0, 1):
                    jj = njj - 2 + c
                    nc.tensor.matmul(
                        avp[:, s0s[c]:], vts[:, jj, h * 128:(h + 1) * 128],
                        ext2_l[:, c, s0s[c]:], start=False, stop=(c == 1))

            def emit_tail(t):
                (h, q0, avp, gacc, ext2_l, s0s, njj) = t
                # exp row-sums: the already-DVE-accumulated groups fold in
                # one ones-matmul; the final group's two chunks ride the PE
                # directly so the last DVE adds leave the critical chain
                sump = psum.tile([1, QCW], F32, tag="pp", bufs=4, name="sump")
                nc.tensor.matmul(sump[:], ones128[:], gacc[:],
                                 start=True, stop=False)
                nc.tensor.matmul(sump[:, s0s[0]:], ones128[:],
                                 ext2_l[:, 0, s0s[0]:],
                                 start=False, stop=False)
                nc.tensor.matmul(sump[:, s0s[1]:], ones128[:],
                                 ext2_l[:, 1, s0s[1]:],
                                 start=False, stop=True)
                # gate*av is independent of the sum — runs off-chain while
                # the reciprocal finishes
                ga = work.tile([128, QCW], BF, tag="ga", bufs=2, name="ga")
                nc.any.tensor_mul(ga[:], avp[:], gts[:, h, q0:q0 + QCW])
                rs = work.tile([1, QCW], BF, tag="rs", bufs=2, name="rs")
                # bf16 1/sum costs ~2^-9 relative on the softmax scale —
                # negligible vs the bf16 inputs — and keeps the broadcast
                # matmul in bf16
                with nc.allow_low_precision(reason="bf16 softmax reciprocal"):
                    nc.vector.reciprocal(out=rs[:], in_=sump[:])
                # broadcast 1/sum across partitions with a K=1 matmul (the
                # PE does it in ~0.2us vs ~1us on the Pool engine)
                bcp = psum.tile([128, QCW], F32, tag="pp", bufs=4, name="bcp")
                nc.tensor.matmul(bcp[:], ones1[:], rs[:], start=True,
                                 stop=True)
                nc.any.tensor_mul(gavs[:, h, q0:q0 + QCW], ga[:], bcp[:])

            for h in range(HPC):
                for qc in range(NQC):
                    q0 = qc * QCW
                    avp = pp("avp")
                    gacc = work.tile([128, QCW], BF, tag="gacc", bufs=2,
                                     name="gacc")
                    njj = 4 * qc + 4
                    G = njj // 2
                    ext2s = [None, None, None]

                    def s0_of(jj):
                        return max(0, (jj - 4 * qc) * 128)

                    def emit_av(g):
                        for c in (0, 1):
                            jj = 2 * g + c
                            s0 = s0_of(jj)
                            nc.tensor.matmul(
                                avp[:, s0:],
                                vts[:, jj, h * 128:(h + 1) * 128],
                                ext2s[g % 3][:, c, s0:],
                                start=(jj == 0), stop=False)

                    for g in range(G):
                        jj0 = 2 * g
                        off0 = jj0 - 4 * qc
                        s00 = s0_of(jj0)
                        sc2 = pp2("sc2")
                        ext2 = work.tile([128, 2, QCW], BF, tag="ext",
                                         bufs=3, name="ext2")
                        ext2s[g % 3] = ext2
                        # both chunks computed from s00 so one wide exp
                        # covers the pair; the sub-diagonal sliver of the
                        # second chunk is zeroed by the mask
                        nc.tensor.matmul(
                            sc2[:, 0, s00:],
                            kts[:, h, jj0 * 128:(jj0 + 1) * 128],
                            qts[:, h, q0 + s00:q0 + QCW],
                            start=True, stop=True)
                        nc.tensor.matmul(
                            sc2[:, 1, s00:],
                            kts[:, h, (jj0 + 1) * 128:(jj0 + 2) * 128],
                            qts[:, h, q0 + s00:q0 + QCW],
                            start=True, stop=True)
                        nc.scalar.activation(ext2[:, :, s00:],
                                             sc2[:, :, s00:],
                                             AF.Exp, scale=SCALE)
                        if off0 >= 0:
                            # diagonal pair: chunk0 keeps q >= its diag,
                            # chunk1 additionally zeroes the 128 columns
                            # below its own diagonal
                            nc.vector.tensor_mul(
                                ext2[:, 0, s00:s00 + 128],
                                ext2[:, 0, s00:s00 + 128], tris[:])
                            nc.vector.tensor_mul(
                                ext2[:, 1, s00:s00 + 256],
                                ext2[:, 1, s00:s00 + 256], ztris[:])
                        # running exp row-sum accumulates on the DVE/ACT so
                        # the PE only pays one ones-matmul per block; the
                        # final group is folded on the PE in the tail
                        if g == 0:
                            nc.any.tensor_copy(out=gacc[:], in_=ext2[:, 0, :])
                        elif g < G - 1:
                            nc.any.tensor_add(gacc[:, s00:], gacc[:, s00:],
                                              ext2[:, 0, s00:])
                        if g < G - 1:
                            s01 = s0_of(jj0 + 1)
                            nc.any.tensor_add(gacc[:, s01:], gacc[:, s01:],
                                              ext2[:, 1, s01:])
                        if pend is not None:
                            emit_tail_av(pend)
                            emit_tail(pend)
                            pend = None
                        if g >= 1:
                            emit_av(g - 1)
                    cur = (h, q0, avp, gacc, ext2s[(G - 1) % 3],
                           (s0_of(njj - 2), s0_of(njj - 1)), njj)
                    if h == 1:
                        # second head: finish the block's tail immediately,
                        # then interleave this seq chunk's o_proj — its
                        # matmuls fill the PE slack of the ACT-paced softmax
                        # and spread the output DMA across the whole phase
                        emit_tail_av(cur)
                        emit_tail(cur)
                        emit_oproj(qc)
                    else:
                        pend = cur
                if h == 0:
                    # flush at the head boundary
                    emit_tail_av(pend)
                    emit_tail(pend)
                    pend = None

    nc.compile()
    return nc


def _prep_inputs(x, q_w, k_w, v_w, o_w, gate_w, gate_b):
    x = np.asarray(x, dtype=np.float32)
    xt = np.ascontiguousarray(x.reshape(S, D).T).astype(BF16)
    gwt = np.ascontiguousarray(np.asarray(gate_w, np.float32).T).astype(BF16)
    gb = np.asarray(gate_b, np.float32).reshape(HD, 1).copy()
    trim = np.triu(np.ones((128, 128), np.float32)).astype(BF16)
    ztrim = np.hstack([np.zeros((128, 128), np.float32),
                       np.triu(np.ones((128, 128), np.float32))]).astype(BF16)
    ow = np.asarray(o_w, np.float32)
    in_maps = []
    for c in range(N_CORES):
        sl = slice(c * E, (c + 1) * E)
        in_maps.append({
            "xt": xt,
            "wqt": np.ascontiguousarray(np.asarray(q_w, np.float32)[sl, :].T).astype(BF16),
            "wkt": np.ascontiguousarray(np.asarray(k_w, np.float32)[sl, :].T).astype(BF16),
            "wvt": np.ascontiguousarray(np.asarray(v_w, np.float32)[sl, :].T).astype(BF16),
            "owt": np.ascontiguousarray(ow[:, sl].T).astype(BF16),
            "gwt": gwt,
            "gb": gb,
            "trim": trim,
            "ztrim": ztrim,
        })
    return in_maps


def _run(in_maps, **kwargs):
    if "nc" not in _CACHED:
        _CACHED["nc"] = _build()
    return run_bass_kernel_spmd(_CACHED["nc"], in_maps,
                                core_ids=list(range(N_CORES)), **kwargs)


def kernel(x, q_w, k_w, v_w, o_w, gate_w, gate_b):
    res = _run(_prep_inputs(x, q_w, k_w, v_w, o_w, gate_w, gate_b))
    y_t = res.results[0]["yt"].astype(np.float32)
    for c in range(1, N_CORES):
        y_t += res.results[c]["yt"].astype(np.float32)
    return np.ascontiguousarray(y_t.T, dtype=np.float32).reshape(1, S, D)


# revision 23
# speedup vs baseline: 1.1933x; 1.1265x over previous
"""GatedAttention Trainium2 kernel, 8-way tensor-parallel over heads.

Reference computation (B=1, S=2048, D=2048, H=16 heads, Hd=128):
  q,k,v = x @ {q,k,v}_w.T  (per-head split)
  scores = (q @ k.T) / sqrt(Hd), causal mask, softmax
  av = attn @ v
  gate = sigmoid(q @ gate_w.T + gate_b)       (per-head)
  y = concat_heads(av * gate) @ o_w.T

Sharding: 2 heads per core (column-parallel QKV/gate). o_proj is computed
as a per-core PARTIAL product over the core's 256 local features into the
full [D, S] output; the host sums the 8 partials. No collectives at all —
each core's program is fully independent, so no core ever waits on
another.

All matmuls run on the PE in bf16 with fp32 PSUM accumulation. Softmax runs
without max-subtraction (scores are small by construction). Attention works
in the transposed [key, query] layout so no on-chip transposes are needed;
exps run 1024 columns at a time (two key chunks share one two-bank PSUM
tile) to halve the ACT instruction count, and exp row-sums accumulate on
the DVE so the PE only pays one ones-matmul per query block. o_proj chunks
are interleaved into the second head's attention stream, filling the PE
slack left by the ACT-paced softmax. A short burst of dummy matmuls at t=0
warms the PE clock gate (HAM) while the first input DMAs are in flight.
"""

import numpy as np
import ml_dtypes

import concourse.bass as bass
import concourse.mybir as mybir
import concourse.tile as tile
from concourse import bacc
from concourse.bass_utils import run_bass_kernel_spmd

BF16 = ml_dtypes.bfloat16
F32 = mybir.dt.float32
BF = mybir.dt.bfloat16
AF = mybir.ActivationFunctionType

N_CORES = 8
S = 2048          # sequence length
D = 2048          # model dim
H = 16            # total heads
HD = 128          # head dim
HPC = H // N_CORES                   # heads per core: 2
E = HPC * HD                         # 256 local features per core
DC = D // 128                        # 16 contraction chunks
QCW = 512                            # q-chunk width
NQC = S // QCW                       # 4 q-chunks
SCALE = 1.0 / float(np.sqrt(HD))

_CACHED = {}


def _build():
    nc = bacc.Bacc("TRN2", target_bir_lowering=False, debug=False,
                   num_devices=1, enable_asserts=False)

    xt = nc.dram_tensor("xt", [D, S], BF, kind="ExternalInput")        # x^T
    wqt = nc.dram_tensor("wqt", [D, E], BF, kind="ExternalInput")      # q_w shard^T
    wkt = nc.dram_tensor("wkt", [D, E], BF, kind="ExternalInput")
    wvt = nc.dram_tensor("wvt", [D, E], BF, kind="ExternalInput")
    owt = nc.dram_tensor("owt", [E, D], BF, kind="ExternalInput")      # o_w cols^T
    gwt = nc.dram_tensor("gwt", [HD, HD], BF, kind="ExternalInput")    # gate_w^T
    gb = nc.dram_tensor("gb", [HD, 1], F32, kind="ExternalInput")      # gate bias
    trim = nc.dram_tensor("trim", [128, 128], BF, kind="ExternalInput")
    ztrim = nc.dram_tensor("ztrim", [128, 256], BF, kind="ExternalInput")
    yt = nc.dram_tensor("yt", [D, S], BF, kind="ExternalOutput")       # y^T partial

    with tile.TileContext(nc) as tc:
        with tc.tile_pool(name="const", bufs=1) as const, \
             tc.tile_pool(name="work", bufs=2) as work, \
             tc.tile_pool(name="psum", bufs=1, space="PSUM") as psum:

            def pp(name):
                return psum.tile([128, QCW], F32, tag="pp", bufs=4, name=name)

            def pp2(name):
                # two contiguous PSUM banks: 1024 score columns per exp
                return psum.tile([128, 2, QCW], F32, tag="sc2", bufs=2,
                                 name=name)

            # ---- PE warm-up: dummy matmuls on a memset tile release the
            #      HAM clock gate (~3.4us of sustained activity) while the
            #      first input DMAs are still in flight ----
            warm = const.tile([128, QCW], BF, tag="warm", name="warm")
            nc.vector.memset(warm[:], 0.0)
            wp = pp("warmp")
            for _ in range(28):
                nc.tensor.matmul(wp[:], warm[:, 0:128], warm[:],
                                 start=True, stop=True)

            # ---- input loads (few big DMAs; xts chunked to feed the
            #      dc-synchronized projection loop) ----
            wqts = const.tile([128, DC, E], BF, tag="wqts", name="wqts")
            wkts = const.tile([128, DC, E], BF, tag="wkts", name="wkts")
            xts = const.tile([128, DC, S], BF, tag="big", name="xts")

            def _ldw(dst, src, half):
                sl = slice(half * 8, (half + 1) * 8)
                nc.sync.dma_start(
                    dst[:, sl, :],
                    src.ap()[half * 1024:(half + 1) * 1024, :]
                       .rearrange("(c p) e -> p c e", p=128))

            # interleave weight halves with the x chunks so the transfer
            # stream stays just ahead of the first pass's dc-ordered
            # consumption
            def _ldx(d0, d1):
                nc.sync.dma_start(
                    xts[:, d0:d1, :],
                    xt.ap()[d0 * 128:d1 * 128, :]
                      .rearrange("(c p) s -> p c s", p=128))

            _ldw(wqts, wqt, 0)
            _ldx(0, 1)
            _ldw(wkts, wkt, 0)
            for d in range(1, 8):
                _ldx(d, d + 1)
            _ldw(wqts, wqt, 1)
            _ldx(8, 9)
            _ldw(wkts, wkt, 1)
            _ldx(9, 10)
            for k in range(5, 8):
                _ldx(2 * k, 2 * k + 2)

            gwts = const.tile([HD, HD], BF, tag="gwts", name="gwts")
            gbs = const.tile([HD, 1], F32, tag="gbs", name="gbs")
            tris = const.tile([128, 128], BF, tag="tris", name="tris")
            ztris = const.tile([128, 256], BF, tag="ztris", name="ztris")
            ones128 = const.tile([128, 1], BF, tag="ones128", name="ones128")
            ones1 = const.tile([1, 128], F32, tag="ones1", name="ones1")
            nc.vector.memset(ones1[:], 1.0)
            nc.sync.dma_start(gwts[:], gwt.ap())
            nc.sync.dma_start(gbs[:], gb.ap())
            nc.sync.dma_start(tris[:], trim.ap())
            nc.sync.dma_start(ztris[:], ztrim.ap())
            nc.vector.memset(ones128[:], 1.0)

            wvts = const.tile([128, DC, E], BF, tag="wvts", name="wvts")
            nc.sync.dma_start(wvts[:], wvt.ap().rearrange("(c p) e -> p c e", p=128))

            # ---- projections ----
            # Q^T, K^T: [e(2x128), s].  Four dc-outer passes of 4 PSUM banks
            # each; pass 1 tracks the streaming xts chunks, later passes
            # re-stream the resident tile.  One weight load per dc per pass.
            qts = const.tile([128, HPC, S], BF, tag="qts", name="qts")
            kts = const.tile([128, HPC, S], BF, tag="kts", name="kts")

            # two dc-outer passes (one per e-chunk); Q accumulates in the 4
            # plain banks, K in the two 2-bank sc2 tiles (idle until
            # attention), so each pass keeps 8 matmuls per dc in flight —
            # enough PE work to track the streaming xts chunks
            for ec in range(HPC):
                qaccs = [pp("qkp") for _ in range(NQC)]
                kaccs = [pp2("ksc") for _ in range(2)]
                for dc in range(DC):
                    st = (dc == 0)
                    sp = (dc == DC - 1)
                    for sc in range(NQC):
                        nc.tensor.matmul(
                            qaccs[sc][:], wqts[:, dc, ec * 128:(ec + 1) * 128],
                            xts[:, dc, sc * QCW:(sc + 1) * QCW],
                            start=st, stop=sp)
                    for sc in range(NQC):
                        nc.tensor.matmul(
                            kaccs[sc // 2][:, sc % 2, :],
                            wkts[:, dc, ec * 128:(ec + 1) * 128],
                            xts[:, dc, sc * QCW:(sc + 1) * QCW],
                            start=st, stop=sp)
                for sc in range(NQC):
                    nc.any.tensor_copy(
                        out=qts[:, ec, sc * QCW:(sc + 1) * QCW],
                        in_=qaccs[sc][:])
                    nc.any.tensor_copy(
                        out=kts[:, ec, sc * QCW:(sc + 1) * QCW],
                        in_=kaccs[sc // 2][:, sc % 2, :])

            # o_proj weights [f(2x128), d]: reuse the wqts slot (dead after
            # the loop above)
            owts = const.tile([128, HPC, D], BF, tag="wqts", name="owts")
            nc.sync.dma_start(owts[:], owt.ap().rearrange("(c p) d -> p c d", p=128))

            # gates for both heads, before the V projection so the sigmoid
            # table load and ACT latency hide behind V's matmuls
            gts = const.tile([128, HPC, S], BF, tag="gts", name="gts")
            for h in range(HPC):
                for qc in range(NQC):
                    gp = pp("gp")
                    nc.tensor.matmul(gp[:], gwts[:],
                                     qts[:, h, qc * QCW:(qc + 1) * QCW],
                                     start=True, stop=True)
                    nc.scalar.activation(gts[:, h, qc * QCW:(qc + 1) * QCW],
                                         gp[:], AF.Sigmoid, bias=gbs[:, 0:1])

            # V: [s(16x128), e] natural layout.  Slot-major (xts is fully
            # resident by now): each psum's 16-matmul chain runs while the
            # previous psum's copy drains, so group boundaries don't stall.
            vts = const.tile([128, DC, E], BF, tag="vts", name="vts")
            for sc16 in range(DC):
                vp = pp("vp")
                for dc in range(DC):
                    nc.tensor.matmul(
                        vp[:, :E],
                        xts[:, dc, sc16 * 128:(sc16 + 1) * 128],
                        wvts[:, dc, :], start=(dc == 0), stop=(dc == DC - 1))
                nc.any.tensor_copy(out=vts[:, sc16, :], in_=vp[:, :E])

            # ---- attention (transposed layout), gated output to SBUF ----
            # gavs[f, s]: the per-head gated, normalized attention output,
            # kept on-chip for the partial o_proj.
            gavs = const.tile([128, HPC, S], BF, tag="gavs", name="gavs")

            def emit_oproj(sc):
                # partial o_proj for one seq chunk:
                # y^T[d, s] = sum_{f local} o_w[d, f] gav[f, s]
                for dc in range(DC):
                    yp = pp("yp")
                    for fc in range(HPC):
                        nc.tensor.matmul(
                            yp[:], owts[:, fc, dc * 128:(dc + 1) * 128],
                            gavs[:, fc, sc * QCW:(sc + 1) * QCW],
                            start=(fc == 0), stop=(fc == HPC - 1))
                    ys = work.tile([128, QCW], BF, tag="ys", bufs=6,
                                   name="ys")
                    nc.any.tensor_copy(out=ys[:], in_=yp[:])
                    nc.sync.dma_start(
                        yt.ap()[dc * 128:(dc + 1) * 128,
                                sc * QCW:(sc + 1) * QCW],
                        ys[:])

            # Software-pipelined across (h, qc) blocks at key-pair (group)
            # granularity: each block's last AV pair and its epilogue are
            # emitted after the NEXT block's first scores/exp, so the PE
            # never idles waiting for the tail exp on ACT.
            pend = None       # deferred tail of the previous block
            pend_fin = None   # its broadcast/normalize half, one group later

            def emit_tail_av(t):
                # deferred AV matmuls for the last group's two chunks
                (h, q0, avp, gacc, ext2_l, s0s, njj) = t
                for c in (0, 1):
                    jj = njj - 2 + c
                    nc.tensor.matmul(
                        avp[:, s0s[c]:], vts[:, jj, h * 128:(h + 1) * 128],
                        ext2_l[:, c, s0s[c]:], start=False, stop=(c == 1))

            def emit_tail_sums(t):
                (h, q0, avp, gacc, ext2_l, s0s, njj) = t
                # exp row-sums land in partition 0 of a bank that is later
                # reused for the broadcast (one ring slot per tail, so the
                # 4-slot rotation never wraps onto a live accumulator).
                # The already-DVE-accumulated groups fold in one ones-matmul;
                # the final group's two chunks ride the PE directly so the
                # last DVE adds leave the critical chain.
                sbp = pp("sbp")
                nc.tensor.matmul(sbp[0:1, :], ones128[:], gacc[:],
                                 start=True, stop=False)
                nc.tensor.matmul(sbp[0:1, s0s[0]:], ones128[:],
                                 ext2_l[:, 0, s0s[0]:],
                                 start=False, stop=False)
                nc.tensor.matmul(sbp[0:1, s0s[1]:], ones128[:],
                                 ext2_l[:, 1, s0s[1]:],
                                 start=False, stop=True)
                # gate*av is independent of the sum — runs off-chain while
                # the reciprocal finishes
                ga = work.tile([128, QCW], BF, tag="ga", bufs=2, name="ga")
                nc.any.tensor_mul(ga[:], avp[:], gts[:, h, q0:q0 + QCW])
                rs = work.tile([1, QCW], F32, tag="rs", bufs=2, name="rs")
                nc.vector.reciprocal_approx_fast(out=rs[:], in_=sbp[0:1, :])
                return (sbp, rs, ga)

            def emit_tail_fin(t, st):
                (h, q0, avp, gacc, ext2_l, s0s, njj) = t
                (sbp, rs, ga) = st
                # broadcast 1/sum across partitions with a K=1 matmul into
                # the same bank the sums came from (WAR-ordered after the
                # reciprocal read)
                nc.tensor.matmul(sbp[:, :], ones1[:], rs[:], start=True,
                                 stop=True)
                nc.any.tensor_mul(gavs[:, h, q0:q0 + QCW], ga[:], sbp[:, :])

            for h in range(HPC):
                for qc in range(NQC):
                    q0 = qc * QCW
                    avp = pp("avp")
                    gacc = work.tile([128, QCW], BF, tag="gacc", bufs=2,
                                     name="gacc")
                    njj = 4 * qc + 4
                    G = njj // 2
                    ext2s = [None, None, None]

                    def s0_of(jj):
                        return max(0, (jj - 4 * qc) * 128)

                    def emit_av(g):
                        for c in (0, 1):
                            jj = 2 * g + c
                            s0 = s0_of(jj)
                            nc.tensor.matmul(
                                avp[:, s0:],
                                vts[:, jj, h * 128:(h + 1) * 128],
                                ext2s[g % 3][:, c, s0:],
                                start=(jj == 0), stop=False)

                    for g in range(G):
                        jj0 = 2 * g
                        off0 = jj0 - 4 * qc
                        s00 = s0_of(jj0)
                        sc2 = pp2("sc2")
                        ext2 = work.tile([128, 2, QCW], BF, tag="ext",
                                         bufs=3, name="ext2")
                        ext2s[g % 3] = ext2
                        # both chunks computed from s00 so one wide exp
                        # covers the pair; the sub-diagonal sliver of the
                        # second chunk is zeroed by the mask
                        nc.tensor.matmul(
                            sc2[:, 0, s00:],
                            kts[:, h, jj0 * 128:(jj0 + 1) * 128],
                            qts[:, h, q0 + s00:q0 + QCW],
                            start=True, stop=True)
                        nc.tensor.matmul(
                            sc2[:, 1, s00:],
                            kts[:, h, (jj0 + 1) * 128:(jj0 + 2) * 128],
                            qts[:, h, q0 + s00:q0 + QCW],
                            start=True, stop=True)
                        nc.scalar.activation(ext2[:, :, s00:],
                                             sc2[:, :, s00:],
                                             AF.Exp, scale=SCALE)
                        if off0 >= 0:
                            # diagonal pair: chunk0 keeps q >= its diag,
                            # chunk1 additionally zeroes the 128 columns
                            # below its own diagonal
                            nc.vector.tensor_mul(
                                ext2[:, 0, s00:s00 + 128],
                                ext2[:, 0, s00:s00 + 128], tris[:])
                            nc.vector.tensor_mul(
                                ext2[:, 1, s00:s00 + 256],
                                ext2[:, 1, s00:s00 + 256], ztris[:])
                        # running exp row-sum accumulates on the DVE/ACT so
                        # the PE only pays one ones-matmul per block; the
                        # final group is folded on the PE in the tail
                        if g == 0:
                            nc.any.tensor_copy(out=gacc[:], in_=ext2[:, 0, :])
                        elif g < G - 1:
                            nc.any.tensor_add(gacc[:, s00:], gacc[:, s00:],
                                              ext2[:, 0, s00:])
                        if g < G - 1:
                            s01 = s0_of(jj0 + 1)
                            nc.any.tensor_add(gacc[:, s01:], gacc[:, s01:],
                                              ext2[:, 1, s01:])
                        if pend is not None:
                            emit_tail_av(pend)
                            pend_fin = (pend, emit_tail_sums(pend))
                            pend = None
                        elif pend_fin is not None:
                            # one group later: the reciprocal has had a full
                            # slot to finish off the PE's critical path
                            emit_tail_fin(*pend_fin)
                            pend_fin = None
                        if g >= 1:
                            emit_av(g - 1)
                    cur = (h, q0, avp, gacc, ext2s[(G - 1) % 3],
                           (s0_of(njj - 2), s0_of(njj - 1)), njj)
                    if h == 1:
                        # second head: finish the block's tail immediately,
                        # then interleave this seq chunk's o_proj — its
                        # matmuls fill the PE slack of the ACT-paced softmax
                        # and spread the output DMA across the whole phase
                        emit_tail_av(cur)
                        st = emit_tail_sums(cur)
                        emit_tail_fin(cur, st)
                        emit_oproj(qc)
                    else:
                        pend = cur
                if h == 0:
                    # flush at the head boundary
                    emit_tail_av(pend)
                    st = emit_tail_sums(pend)
                    emit_tail_fin(pend, st)
                    pend = None

    nc.compile()
    return nc


def _prep_inputs(x, q_w, k_w, v_w, o_w, gate_w, gate_b):
    x = np.asarray(x, dtype=np.float32)
    xt = np.ascontiguousarray(x.reshape(S, D).T).astype(BF16)
    gwt = np.ascontiguousarray(np.asarray(gate_w, np.float32).T).astype(BF16)
    gb = np.asarray(gate_b, np.float32).reshape(HD, 1).copy()
    trim = np.triu(np.ones((128, 128), np.float32)).astype(BF16)
    ztrim = np.hstack([np.zeros((128, 128), np.float32),
                       np.triu(np.ones((128, 128), np.float32))]).astype(BF16)
    ow = np.asarray(o_w, np.float32)
    in_maps = []
    for c in range(N_CORES):
        sl = slice(c * E, (c + 1) * E)
        in_maps.append({
            "xt": xt,
            "wqt": np.ascontiguousarray(np.asarray(q_w, np.float32)[sl, :].T).astype(BF16),
            "wkt": np.ascontiguousarray(np.asarray(k_w, np.float32)[sl, :].T).astype(BF16),
            "wvt": np.ascontiguousarray(np.asarray(v_w, np.float32)[sl, :].T).astype(BF16),
            "owt": np.ascontiguousarray(ow[:, sl].T).astype(BF16),
            "gwt": gwt,
            "gb": gb,
            "trim": trim,
            "ztrim": ztrim,
        })
    return in_maps


def _run(in_maps, **kwargs):
    if "nc" not in _CACHED:
        _CACHED["nc"] = _build()
    return run_bass_kernel_spmd(_CACHED["nc"], in_maps,
                                core_ids=list(range(N_CORES)), **kwargs)


def kernel(x, q_w, k_w, v_w, o_w, gate_w, gate_b):
    res = _run(_prep_inputs(x, q_w, k_w, v_w, o_w, gate_w, gate_b))
    y_t = res.results[0]["yt"].astype(np.float32)
    for c in range(1, N_CORES):
        y_t += res.results[c]["yt"].astype(np.float32)
    return np.ascontiguousarray(y_t.T, dtype=np.float32).reshape(1, S, D)
